# revision 4
# baseline (speedup 1.0000x reference)
"""Conv4D (3^4 taps, SAME, stride 1) + bias, scaled by 1/sqrt(2).

Strategy: data-parallel over batch (B=8 -> 8 NeuronCores), weights replicated.
Per core the conv is an implicit GEMM on the TensorEngine:
  contraction K = (k4-tap, Cin) = 3*32 = 96  -- the (z, ci) axis is contiguous
    in the channels-last layout, so the z-window "im2col" is just a 96-wide
    slice of the flattened (z*ci) axis;
  stationary = W tap [96, 64], moving = x window [96, 512] (two w-planes of
    one 16x16 (x,y) tile), PSUM accumulates the 27 remaining (k1,k2,k3) taps.
x is zero-padded on the host in w/x/y/z so every tap is a full rectangle and
every DMA collapses to a 2D contiguous-stride transfer.  Matmuls run in
float32r (~13-bit mantissa, ~4x fp32 throughput, rel err ~1e-4); operands are
rounded to f32r on the VectorEngine as required by the BIR verifier.
"""

import numpy as np

import concourse.bacc as bacc
import concourse.bass as bass
import concourse.mybir as mybir
import concourse.tile as tile
from concourse.bass_utils import run_bass_kernel_spmd

INV_SQRT2 = 0.7071067811865476

B = 8            # batch, one element per core
S = 16           # spatial extent in each of the 4 dims
SP = S + 2       # padded extent
CIN = 32
COUT = 64
KT = 3           # taps per dim
ZCP = SP * CIN   # padded flattened (z, ci) axis = 576
KP = KT * CIN    # contraction size per matmul = 96
NP = 8           # w-plane pairs per core

_cached = {}


def _build_nc():
    f32 = mybir.dt.float32
    f32r = mybir.dt.float32r
    nc = bacc.Bacc("TRN2", target_bir_lowering=False, debug=False, num_devices=B)

    x_d = nc.dram_tensor("x", (SP, SP, SP, ZCP), f32, kind="ExternalInput")
    w_d = nc.dram_tensor("w", (KT * KT * KT, KP, COUT), f32, kind="ExternalInput")
    b_d = nc.dram_tensor("bscaled", (COUT, 1), f32, kind="ExternalInput")
    o_d = nc.dram_tensor("out", (S, S, S, S, COUT), f32, kind="ExternalOutput")

    taps = [(k1, k2, k3) for k1 in range(KT) for k2 in range(KT) for k3 in range(KT)]

    with tile.TileContext(nc) as tc:
        with (
            tc.tile_pool(name="wpool", bufs=1) as wpool,
            tc.tile_pool(name="xpool", bufs=3) as xpool,
            tc.tile_pool(name="xrpool", bufs=10) as xrpool,
            tc.tile_pool(name="opool", bufs=4) as opool,
            tc.tile_pool(name="ppool", bufs=4, space=bass.MemorySpace.PSUM) as ppool,
        ):
            wt_f = wpool.tile([KP, KT * KT * KT, COUT], f32)
            nc.sync.dma_start(wt_f[:], w_d[:].transpose([1, 0, 2]))
            wt = wpool.tile([KP, KT * KT * KT, COUT], f32r)
            nc.vector.tensor_copy(wt[:], wt_f[:])
            bt = wpool.tile([COUT, 1], f32)
            nc.sync.dma_start(bt[:], b_d[:])

            for z in range(S):
                xrs = []
                for p in range(NP):
                    xt = xpool.tile([KP, 4, SP, SP], f32)
                    nc.sync.dma_start(
                        xt[:],
                        x_d[2 * p : 2 * p + 4, :, :, z * CIN : z * CIN + KP]
                        .transpose([3, 0, 1, 2]),
                    )
                    xr = xrpool.tile([KP, 4, SP, SP], f32r)
                    nc.vector.tensor_copy(xr[:], xt[:])
                    xrs.append(xr)

                for p in range(NP):
                    pt = ppool.tile([COUT, 2, S, S], f32)
                    for i, (k1, k2, k3) in enumerate(taps):
                        nc.tensor.matmul(
                            pt[:],
                            wt[:, (k1 * KT + k2) * KT + k3, :],
                            xrs[p][:, k1 : k1 + 2, k2 : k2 + S, k3 : k3 + S],
                            start=(i == 0),
                            stop=(i == len(taps) - 1),
                        )
                    ot = opool.tile([COUT, 2, S, S], f32)
                    nc.scalar.activation(
                        ot[:], pt[:],
                        mybir.ActivationFunctionType.Identity,
                        bias=bt[:],
                        scale=INV_SQRT2,
                    )
                    nc.sync.dma_start(
                        o_d[2 * p : 2 * p + 2, :, :, z, :].transpose([3, 0, 1, 2]),
                        ot[:],
                    )

    nc.compile()
    return nc


def kernel(x, W, b):
    if "nc" not in _cached:
        _cached["nc"] = _build_nc()
    nc = _cached["nc"]

    x = np.asarray(x, dtype=np.float32)
    xp = np.zeros((B, SP, SP, SP, ZCP), dtype=np.float32)
    xp[:, 1 : S + 1, 1 : S + 1, 1 : S + 1, CIN : CIN + S * CIN] = x.reshape(
        B, S, S, S, S * CIN
    )
    wr = np.ascontiguousarray(
        np.asarray(W, dtype=np.float32).reshape(KT * KT * KT, KP, COUT)
    )
    br = np.ascontiguousarray(
        (np.asarray(b, dtype=np.float32) * INV_SQRT2).reshape(COUT, 1)
    )

    in_maps = [{"x": xp[i], "w": wr, "bscaled": br} for i in range(B)]
    res = run_bass_kernel_spmd(nc, in_maps, core_ids=list(range(B)))
    kernel.last_exec_time_ns = res.exec_time_ns
    out = np.stack([res.results[i]["out"] for i in range(B)], axis=0)
    return out.reshape(B, S, S, S, S, COUT)


kernel.last_exec_time_ns = None
